# revision 26
# baseline (speedup 1.0000x reference)
"""Trainium2 Bass kernel for nn_MultiHeadHCGAttention.

Math notes (exact restructuring of the reference):
  The key_padding_mask replaces the ENTIRE key feature row with -1e9 BEFORE
  the K projection (v is NOT masked). Hence every masked key position s in
  batch b has the SAME projected K row:
      Kmask[n] = -1e9 * sum_h Wk[n,h,:] + bk[n]   (data independent)
  All masked keys share one score z = Q.Kmask/sqrt(dk) with |z| ~ 1e9.
  In fp32 softmax the output per (query q, head n) is therefore either
    - mean of V over the masked key positions  if z > max unmasked score
      (uniform softmax over the identical-score masked keys)
    - standard softmax over unmasked keys      otherwise (masked weights
      underflow to exactly 0 in fp32)
  The boundary band has probability ~1e-7 per query -> decided by sign(z),
  computed exactly on the host in fp64 (z = q @ (Wq@Kmask) + bq.Kmask).

  KEY SPARSITY EXPLOITED ON DEVICE: ~50% of (query, head) pairs take the
  mask branch; their output is entirely host-computed (ubar via ycorr).
  The device only processes, per head, the GATHERED not-chosen queries
  (like the keys are gathered per batch) — roughly halving all
  query-dimension work (q-projection, scores, exp, PV, out-projection).
  Because the gathered query sets differ per head, the output projection
  is per-head; the host scatter-adds the per-head partials into the full
  output (and adds bo + ycorr in fp64/fp32).

Sharding: 8 cores = (batch b in 0..3) x (head group of 4). No collectives.

Softmax denominator: e-tiles are accumulated over key-tiles on the vector
engine (esum), one ones-matmul per (head, query-chunk) on the PE, and
1/d via reciprocal_approx_fast (~18-bit).
"""

import math
import sys

if "/opt/trn_rl_repo" not in sys.path:
    sys.path.insert(0, "/opt/trn_rl_repo")

import ml_dtypes
import numpy as np

import concourse.bacc as bacc
import concourse.tile as tile
from concourse import mybir
from concourse.bass_utils import run_bass_kernel_spmd

S, B, H = 2048, 4, 1024
NH, DK = 8, 128
NHDK = NH * DK
NHL = 4            # heads per core (head group)
GW = NHL * DK      # 512: projection width per core
NEG = -1.0e9
NCORES = 8
HT = H // 128      # 8 H-tiles

bf16 = mybir.dt.bfloat16
f32 = mybir.dt.float32
npbf16 = ml_dtypes.bfloat16

_PROG_CACHE: dict = {}


def _chunks(total, step=512):
    out = []
    o = 0
    while o < total:
        w = min(step, total - o)
        out.append((o, w))
        o += w
    return out


def build_program(UP: int, UNP: int):
    """Per-core SPMD program.

    UP  = gathered unmasked-key count (global max over batches)
    UNP = gathered not-chosen query slot width per head (global max)
    """
    NKT = (UP + 127) // 128
    ktiles = [(o, min(128, UP - o)) for o in range(0, UP, 128)]
    kchunks = _chunks(UP)
    qchunks = _chunks(UNP)
    NQC = len(qchunks)
    assert NKT >= 3  # esum init uses tiles kt=0,1 at full width

    nc = bacc.Bacc("TRN2", target_bir_lowering=False, debug=False)

    d_qg = nc.dram_tensor("qg", [H, NHL * UNP], bf16, kind="ExternalInput")
    d_kuT = nc.dram_tensor("kuT", [H, UP], bf16, kind="ExternalInput")
    d_vuT = nc.dram_tensor("vuT", [H, UP], bf16, kind="ExternalInput")
    d_wq = nc.dram_tensor("wq", [H, GW], bf16, kind="ExternalInput")
    d_wk = nc.dram_tensor("wk", [H, GW], bf16, kind="ExternalInput")
    d_wv = nc.dram_tensor("wv", [H, GW], bf16, kind="ExternalInput")
    d_wo = nc.dram_tensor("wo", [GW, H], bf16, kind="ExternalInput")
    d_bq = nc.dram_tensor("bq", [DK, NHL], f32, kind="ExternalInput")
    d_bk = nc.dram_tensor("bk", [DK, NHL], f32, kind="ExternalInput")
    d_bv = nc.dram_tensor("bv", [1, GW], bf16, kind="ExternalInput")
    d_padb = nc.dram_tensor("padb", [128, NKT], f32, kind="ExternalInput")
    d_y = nc.dram_tensor("y", [H, NHL * UNP], bf16, kind="ExternalOutput")

    SCALE = 1.0 / math.sqrt(DK)

    with tile.TileContext(nc) as tc:
        with (
            tc.tile_pool(name="const", bufs=1) as const,
            tc.tile_pool(name="kv", bufs=1) as kvp,
            tc.tile_pool(name="qg", bufs=3) as qgp,
            tc.tile_pool(name="qs", bufs=3) as qsp,
            tc.tile_pool(name="vg", bufs=1) as vgp,
            tc.tile_pool(name="exp", bufs=10) as expp,
            tc.tile_pool(name="es", bufs=2) as esp,
            tc.tile_pool(name="sc", bufs=2) as scp,
            tc.tile_pool(name="oc", bufs=3) as ocp,
            tc.tile_pool(name="yt", bufs=2) as ytp,
            tc.tile_pool(name="ps_pj", bufs=2, space="PSUM") as ps_pj,
            tc.tile_pool(name="ps_oj", bufs=1, space="PSUM") as ps_oj,
            tc.tile_pool(name="ps_sc", bufs=2, space="PSUM") as ps_sc,
            tc.tile_pool(name="ps_pv", bufs=2, space="PSUM") as ps_pv,
            tc.tile_pool(name="ps_pd", bufs=1, space="PSUM") as ps_pd,
        ):
            # ---- SBUF residents ----
            kuT = const.tile([128, HT, UP], bf16)
            vuT = const.tile([128, HT, UP], bf16)
            wq = const.tile([128, HT, GW], bf16)
            wk = const.tile([128, HT, GW], bf16)
            wv = const.tile([128, HT, GW], bf16)
            wo = const.tile([128, NHL, H], bf16)
            bq = const.tile([128, NHL], f32)
            bk = const.tile([128, NHL], f32)
            padb = const.tile([128, NKT], f32)
            ones_mat = const.tile([128, 128], bf16)
            nc.vector.memset(ones_mat[:], 1.0)

            r_qg = d_qg[:].rearrange("(t p) u -> p t u", p=128)
            r_y = d_y[:].rearrange("(t p) u -> p t u", p=128)
            r_kuT = d_kuT[:].rearrange("(t p) u -> p t u", p=128)
            r_vuT = d_vuT[:].rearrange("(t p) u -> p t u", p=128)
            r_wq = d_wq[:].rearrange("(t p) d -> p t d", p=128)
            r_wk = d_wk[:].rearrange("(t p) d -> p t d", p=128)
            r_wv = d_wv[:].rearrange("(t p) d -> p t d", p=128)
            r_wo = d_wo[:].rearrange("(n p) h -> p n h", p=128)

            # qg streamed per (head, chunk) just-in-time
            def load_qg(n, ch):
                o, w = qchunks[ch]
                t = qgp.tile([128, HT, 512], bf16, tag="qg", name="qg")
                nc.sync.dma_start(
                    t[:, :, :w], r_qg[:, :, n * UNP + o : n * UNP + o + w]
                )
                return t

            # ---- input DMA ring (single SP HWDGE queue, FIFO) ----
            nc.sync.dma_start(bq[:], d_bq[:])
            nc.sync.dma_start(bk[:], d_bk[:])
            nc.sync.dma_start(padb[:], d_padb[:])
            for ht in range(HT):
                nc.sync.dma_start(wk[:, ht, :], r_wk[:, ht, :])
                nc.sync.dma_start(kuT[:, ht, :], r_kuT[:, ht, :])
            for ht in range(HT):
                nc.sync.dma_start(wq[:, ht, :], r_wq[:, ht, :])
            qg_tiles = {}
            qg_tiles[(0, 0)] = load_qg(0, 0)
            for n in range(NHL):
                nc.sync.dma_start(wo[:, n, :], r_wo[:, n, :])
            for ht in range(HT):
                nc.sync.dma_start(vuT[:, ht, :], r_vuT[:, ht, :])
                nc.sync.dma_start(wv[:, ht, :], r_wv[:, ht, :])
            # remaining qg chunks stream in consumption order below

            bvb = const.tile([128, GW], bf16)
            nc.gpsimd.dma_start(bvb[:], d_bv[0:1, :].to_broadcast([128, GW]))

            def kproj(n):
                ksb = kvp.tile([128, UP], bf16, tag=f"ksb{n}", name=f"ksb{n}")
                for o, w in kchunks:
                    pk = ps_pj.tile([128, 512], f32, tag="pj", name="pk")
                    for ht in range(HT):
                        nc.tensor.matmul(
                            pk[:, :w],
                            wk[:, ht, n * 128 : (n + 1) * 128],
                            kuT[:, ht, o : o + w],
                            start=(ht == 0),
                            stop=(ht == HT - 1),
                        )
                    nc.vector.tensor_scalar_add(
                        ksb[:, o : o + w], pk[:, :w], bk[:, n : n + 1]
                    )
                return ksb

            def vproj():
                vg = vgp.tile([128, NKT, GW], bf16)
                for kt, (ko, klen) in enumerate(ktiles):
                    pv = ps_pj.tile([128, 512], f32, tag="pj", name="pv")
                    for ht in range(HT):
                        nc.tensor.matmul(
                            pv[:klen],
                            vuT[:, ht, ko : ko + klen],
                            wv[:, ht, :],
                            start=(ht == 0),
                            stop=(ht == HT - 1),
                        )
                    nc.vector.tensor_add(vg[:klen, kt, :], pv[:klen], bvb[:klen])
                return vg

            def qproj(n, ch):
                """Project gathered queries for (head n, chunk ch)."""
                o, w = qchunks[ch]
                qgt = qg_tiles.pop((n, ch))
                qsb = qsp.tile([128, 512], bf16, tag="qsb", name="qsb")
                pq = ps_pj.tile([128, 512], f32, tag="pj", name="pq")
                for ht in range(HT):
                    nc.tensor.matmul(
                        pq[:, :w],
                        wq[:, ht, n * 128 : (n + 1) * 128],
                        qgt[:, ht, :w],
                        start=(ht == 0),
                        stop=(ht == HT - 1),
                    )
                nc.vector.tensor_scalar_add(qsb[:, :w], pq[:, :w], bq[:, n : n + 1])
                return qsb

            def attention(n, ch, ksb, qsb, vg):
                """Attention + per-head output projection for one q-chunk."""
                o, w = qchunks[ch]
                ppv = ps_pv.tile([128, 512], f32, name="ppv")
                # esum split into two independent partial chains: low kt
                # tiles accumulate on DVE, high kt tiles on the (otherwise
                # idle) gpsimd engine; one DVE add merges them. Only the
                # first klen partitions of each e are valid — slice by klen
                # so the short last tile doesn't pull in stale pool data.
                esum = esp.tile([128, 512], bf16, tag="esum", name="esum")
                esumB = esp.tile([128, 512], bf16, tag="esumB", name="esumB")
                kt_split = (NKT + 1) // 2  # group0: [0, split), group1: rest
                # group1's first two tiles must be full-width (the only
                # partial tile is the last one)
                assert kt_split >= 2 and NKT - kt_split >= 3
                etiles = []
                for kt, (ko, klen) in enumerate(ktiles):
                    ps = ps_sc.tile([128, 512], f32, tag="sc", name="ps")
                    nc.tensor.matmul(
                        ps[:klen, :w],
                        ksb[:, ko : ko + klen],
                        qsb[:, :w],
                        start=True,
                        stop=True,
                    )
                    e = expp.tile([128, 512], bf16, name="e")
                    nc.scalar.activation(
                        out=e[:klen, :w],
                        in_=ps[:klen, :w],
                        func=mybir.ActivationFunctionType.Exp,
                        bias=padb[:klen, kt : kt + 1],
                        scale=SCALE,
                    )
                    nc.tensor.matmul(
                        ppv[:, :w],
                        vg[:klen, kt, n * 128 : (n + 1) * 128],
                        e[:klen, :w],
                        start=(kt == 0),
                        stop=(kt == NKT - 1),
                    )
                    etiles.append((e, klen))
                    grp = 0 if kt < kt_split else 1
                    acc, eng = (esum, nc.vector) if grp == 0 else (esumB, nc.gpsimd)
                    first = 0 if grp == 0 else kt_split
                    if kt == first:
                        pass
                    elif kt == first + 1:
                        ep, kp = etiles[first]
                        kl = min(klen, kp)
                        eng.tensor_add(acc[:kl, :w], ep[:kl, :w], e[:kl, :w])
                    else:
                        eng.tensor_add(acc[:klen, :w], acc[:klen, :w], e[:klen, :w])
                nc.vector.tensor_add(esum[:, :w], esum[:, :w], esumB[:, :w])
                pd = ps_pd.tile([128, 512], f32, name="pd")
                nc.tensor.matmul(pd[:, :w], ones_mat[:], esum[:, :w], start=True, stop=True)
                rec = scp.tile([128, 512], f32, tag="rec", name="rec")
                nc.vector.reciprocal_approx_fast(out=rec[:, :w], in_=pd[:, :w])
                oc = ocp.tile([128, 512], bf16, name="oc")
                nc.vector.tensor_mul(oc[:, :w], ppv[:, :w], rec[:, :w])

                # per-head output projection for this chunk
                yt = ytp.tile([128, HT, 512], bf16, name="yt")
                for ht in range(HT):
                    py = ps_oj.tile([128, 512], f32, tag="oj", name="py")
                    nc.tensor.matmul(
                        py[:, :w],
                        wo[:, n, ht * 128 : (ht + 1) * 128],
                        oc[:, :w],
                        start=True,
                        stop=True,
                    )
                    if ht in (2, 5, 7):
                        nc.vector.tensor_scalar_add(yt[:, ht, :w], py[:, :w], 0.0)
                    else:
                        nc.scalar.copy(yt[:, ht, :w], py[:, :w])
                nc.gpsimd.dma_start(
                    r_y[:, :, n * UNP + o : n * UNP + o + w], yt[:, :, :w]
                )

            # ---- emission ----
            ks = [kproj(n) for n in range(NHL)]
            qs0 = qproj(0, 0)
            vg = vproj()
            units = [(n, ch) for n in range(NHL) for ch in range(NQC)]
            qsbs = {(0, 0): qs0}
            # stream remaining qg loads in consumption order on the ring
            for u in units[1:]:
                qg_tiles[u] = load_qg(*u)
            for i, (n, ch) in enumerate(units):
                # prefetch the NEXT unit's q-projection BEFORE this unit's
                # attention so its scores can overlap this unit's finalize
                # and output projection
                if i + 1 < len(units):
                    nxt = units[i + 1]
                    qsbs[nxt] = qproj(*nxt)
                attention(n, ch, ks[n], qsbs.pop((n, ch)), vg)

    nc.compile()
    return nc


def _prepare(query, key, value, key_padding_mask, Wq, bq, Wk, bk, Wv, bv, Wo, bo):
    """Host-side prep: mask constants (fp64), gathers/transposes, per-core maps."""
    mask = np.asarray(key_padding_mask)
    q64 = np.asarray(query, np.float64)
    Wq64 = np.asarray(Wq, np.float64)
    Wk64 = np.asarray(Wk, np.float64)
    Wv64 = np.asarray(Wv, np.float64)
    Wo64 = np.asarray(Wo, np.float64)

    # shared projected row of all masked keys, per head
    kmask = NEG * Wk64.sum(axis=1) + np.asarray(bk, np.float64)  # [NH, DK]

    # z sign per (s, b, n):  z = q . (Wq[n] @ kmask[n]) + bq[n].kmask[n]
    wz = np.einsum("nhd,nd->hn", Wq64, kmask)  # [H, NH]
    cz = np.einsum("nd,nd->n", np.asarray(bq, np.float64), kmask)  # [NH]
    z = q64.reshape(S * B, H) @ wz + cz  # [S*B, NH]
    choose = (z > 0).reshape(S, B, NH)

    # mask-branch output: mean of (unmasked-data) V over masked key positions
    v64 = np.asarray(value, np.float64)  # [S, B, H]
    vbar_feat = np.stack(
        [
            v64[mask[b], b, :].mean(axis=0) if mask[b].any() else np.zeros(H)
            for b in range(B)
        ]
    )  # [B, H]
    for b in range(B):
        if not mask[b].any():
            choose[:, b, :] = False  # no masked keys -> no mask branch
        elif mask[b].all():
            # all keys masked: identical scores -> uniform softmax -> Vbar
            choose[:, b, :] = True
    vbar = (
        np.einsum("bh,nhd->bnd", vbar_feat, Wv64) + np.asarray(bv, np.float64)[None]
    )  # [B, NH, DK]
    ubar = np.einsum("bnd,ndh->bnh", vbar, Wo64.reshape(NH, DK, H))  # [B, NH, H]

    # correction added on host for mask-branch rows
    ycorr = np.einsum("sbn,bnh->sbh", choose.astype(np.float64), ubar)

    # gather unmasked keys per batch
    idx = [np.nonzero(~mask[b])[0] for b in range(B)]
    UP = max(max(len(i) for i in idx), 1)
    NKT = (UP + 127) // 128

    # gather not-chosen queries per (batch, head)
    qidx = [[np.nonzero(~choose[:, b, n])[0] for n in range(NH)] for b in range(B)]
    UNP = max(max(len(qidx[b][n]) for n in range(NH)) for b in range(B))
    UNP = max(UNP, 1)

    Wq_np = np.asarray(Wq, np.float32)
    Wk_np = np.asarray(Wk, np.float32)
    Wv_np = np.asarray(Wv, np.float32)
    Wo_np = np.asarray(Wo, np.float32)
    bq_np = np.asarray(bq, np.float32)
    bk_np = np.asarray(bk, np.float32)
    bv_np = np.asarray(bv, np.float32)
    bo_np = np.asarray(bo, np.float32)

    base_padb = np.zeros((128, NKT), np.float32)
    flat = np.arange(NKT * 128).reshape(NKT, 128).T  # [128, NKT] key index

    q_np = np.asarray(query, np.float32)

    kuT_b, vuT_b, padb_b = [], [], []
    for b in range(B):
        ii = idx[b]
        u = len(ii)
        kuT = np.zeros((H, UP), npbf16)
        kuT[:, :u] = np.asarray(key[ii, b, :], np.float32).T.astype(npbf16)
        vuT = np.zeros((H, UP), npbf16)
        vuT[:, :u] = np.asarray(value[ii, b, :], np.float32).T.astype(npbf16)
        pb = base_padb.copy()
        pb[flat >= max(u, 1)] = -30000.0  # keep >=1 live key (denom > 0)
        kuT_b.append(kuT)
        vuT_b.append(vuT)
        padb_b.append(pb)

    in_maps = []
    for core in range(NCORES):
        b, g = divmod(core, 2)
        hsl = slice(g * NHL, (g + 1) * NHL)
        qg = np.zeros((H, NHL * UNP), npbf16)
        for nl in range(NHL):
            qi = qidx[b][g * NHL + nl]
            qg[:, nl * UNP : nl * UNP + len(qi)] = (
                q_np[qi, b, :].T.astype(npbf16)
            )
        wq_d = np.ascontiguousarray(
            Wq_np[hsl].transpose(1, 0, 2).reshape(H, GW)
        ).astype(npbf16)
        wk_d = np.ascontiguousarray(
            Wk_np[hsl].transpose(1, 0, 2).reshape(H, GW)
        ).astype(npbf16)
        wv_d = np.ascontiguousarray(
            Wv_np[hsl].transpose(1, 0, 2).reshape(H, GW)
        ).astype(npbf16)
        wo_d = np.ascontiguousarray(Wo_np[g * GW : (g + 1) * GW, :]).astype(npbf16)
        bq_d = np.ascontiguousarray(bq_np[hsl].T)  # [DK, NHL]
        bk_d = np.ascontiguousarray(bk_np[hsl].T)
        bv_d = bv_np[hsl].reshape(1, GW).astype(npbf16)
        in_maps.append(
            {
                "qg": qg,
                "kuT": kuT_b[b],
                "vuT": vuT_b[b],
                "wq": wq_d,
                "wk": wk_d,
                "wv": wv_d,
                "wo": wo_d,
                "bq": bq_d,
                "bk": bk_d,
                "bv": bv_d,
                "padb": padb_b[b],
            }
        )
    return in_maps, ycorr + bo_np[None, None, :], UP, UNP, qidx


def run(inputs: dict, trace: bool = False):
    in_maps, ycorr, UP, UNP, qidx = _prepare(**inputs)
    key_ = (UP, UNP)
    if key_ not in _PROG_CACHE:
        _PROG_CACHE[key_] = build_program(UP, UNP)
    nc = _PROG_CACHE[key_]
    res = run_bass_kernel_spmd(nc, in_maps, list(range(NCORES)), trace=trace)
    y = np.ascontiguousarray(ycorr.astype(np.float32))  # [S, B, H], incl. bo
    for core in range(NCORES):
        b, g = divmod(core, 2)
        yp = res.results[core]["y"].astype(np.float32)  # [H, NHL*UNP]
        for nl in range(NHL):
            qi = qidx[b][g * NHL + nl]
            if len(qi):
                y[qi, b, :] += yp[:, nl * UNP : nl * UNP + len(qi)].T
    return y, res


def kernel(**inputs) -> np.ndarray:
    y, _ = run(inputs, trace=False)
    return y


# revision 29
# speedup vs baseline: 1.0558x; 1.0558x over previous
"""Trainium2 Bass kernel for nn_MultiHeadHCGAttention.

Math notes (exact restructuring of the reference):
  The key_padding_mask replaces the ENTIRE key feature row with -1e9 BEFORE
  the K projection (v is NOT masked). Hence every masked key position s in
  batch b has the SAME projected K row:
      Kmask[n] = -1e9 * sum_h Wk[n,h,:] + bk[n]   (data independent)
  All masked keys share one score z = Q.Kmask/sqrt(dk) with |z| ~ 1e9.
  In fp32 softmax the output per (query q, head n) is therefore either
    - mean of V over the masked key positions  if z > max unmasked score
      (uniform softmax over the identical-score masked keys)
    - standard softmax over unmasked keys      otherwise (masked weights
      underflow to exactly 0 in fp32)
  The boundary band has probability ~1e-7 per query -> decided by sign(z),
  computed exactly on the host in fp64 (z = q @ (Wq@Kmask) + bq.Kmask).

  KEY SPARSITY EXPLOITED ON DEVICE: ~50% of (query, head) pairs take the
  mask branch; their output is entirely host-computed (ubar via ycorr).
  The device only processes, per head, the GATHERED not-chosen queries
  (like the keys are gathered per batch) — roughly halving all
  query-dimension work (q-projection, scores, exp, PV, out-projection).
  Because the gathered query sets differ per head, the output projection
  is per-head; the host scatter-adds the per-head partials into the full
  output (and adds bo + ycorr in fp64/fp32).

Sharding: 8 cores = (batch b in 0..3) x (head group of 4). No collectives.

Softmax denominator: e-tiles are accumulated over key-tiles on the vector
engine (esum), one ones-matmul per (head, query-chunk) on the PE, and
1/d via reciprocal_approx_fast (~18-bit).
"""

import math
import sys

if "/opt/trn_rl_repo" not in sys.path:
    sys.path.insert(0, "/opt/trn_rl_repo")

import ml_dtypes
import numpy as np

import concourse.bacc as bacc
import concourse.tile as tile
from concourse import mybir
from concourse.bass_utils import run_bass_kernel_spmd

S, B, H = 2048, 4, 1024
NH, DK = 8, 128
NHDK = NH * DK
NHL = 4            # heads per core (head group)
GW = NHL * DK      # 512: projection width per core
NEG = -1.0e9
NCORES = 8
HT = H // 128      # 8 H-tiles

bf16 = mybir.dt.bfloat16
f32 = mybir.dt.float32
npbf16 = ml_dtypes.bfloat16

_PROG_CACHE: dict = {}


def _chunks(total, step=512):
    out = []
    o = 0
    while o < total:
        w = min(step, total - o)
        out.append((o, w))
        o += w
    return out


def build_program(UP: int, UNP: int):
    """Per-core SPMD program.

    UP  = gathered unmasked-key count (global max over batches)
    UNP = gathered not-chosen query slot width per head (global max)
    """
    NKT = (UP + 127) // 128
    ktiles = [(o, min(128, UP - o)) for o in range(0, UP, 128)]
    kchunks = _chunks(UP)
    qchunks = _chunks(UNP)
    NQC = len(qchunks)
    assert NKT >= 3  # esum init uses tiles kt=0,1 at full width

    nc = bacc.Bacc("TRN2", target_bir_lowering=False, debug=False)

    d_qg = nc.dram_tensor("qg", [H, NHL * UNP], bf16, kind="ExternalInput")
    d_kuT = nc.dram_tensor("kuT", [H, UP], bf16, kind="ExternalInput")
    d_vuT = nc.dram_tensor("vuT", [H, UP], bf16, kind="ExternalInput")
    d_wq = nc.dram_tensor("wq", [H, GW], bf16, kind="ExternalInput")
    d_wk = nc.dram_tensor("wk", [H, GW], bf16, kind="ExternalInput")
    d_wv = nc.dram_tensor("wv", [H, GW], bf16, kind="ExternalInput")
    d_wo = nc.dram_tensor("wo", [GW, H], bf16, kind="ExternalInput")
    d_bq = nc.dram_tensor("bq", [DK, NHL], f32, kind="ExternalInput")
    d_bk = nc.dram_tensor("bk", [DK, NHL], f32, kind="ExternalInput")
    d_bv = nc.dram_tensor("bv", [1, GW], bf16, kind="ExternalInput")
    d_padb = nc.dram_tensor("padb", [128, NKT], f32, kind="ExternalInput")
    d_y = nc.dram_tensor("y", [H, NHL * UNP], bf16, kind="ExternalOutput")

    SCALE = 1.0 / math.sqrt(DK)

    with tile.TileContext(nc) as tc:
        with (
            tc.tile_pool(name="const", bufs=1) as const,
            tc.tile_pool(name="kv", bufs=1) as kvp,
            tc.tile_pool(name="qg", bufs=3) as qgp,
            tc.tile_pool(name="qs", bufs=3) as qsp,
            tc.tile_pool(name="vg", bufs=1) as vgp,
            tc.tile_pool(name="exp", bufs=10) as expp,
            tc.tile_pool(name="es", bufs=2) as esp,
            tc.tile_pool(name="sc", bufs=2) as scp,
            tc.tile_pool(name="oc", bufs=3) as ocp,
            tc.tile_pool(name="yt", bufs=2) as ytp,
            tc.tile_pool(name="ps_pj", bufs=2, space="PSUM") as ps_pj,
            tc.tile_pool(name="ps_oj", bufs=1, space="PSUM") as ps_oj,
            tc.tile_pool(name="ps_sc", bufs=2, space="PSUM") as ps_sc,
            tc.tile_pool(name="ps_pv", bufs=2, space="PSUM") as ps_pv,
            tc.tile_pool(name="ps_pd", bufs=1, space="PSUM") as ps_pd,
        ):
            # ---- SBUF residents ----
            kuT = const.tile([128, HT, UP], bf16)
            vuT = const.tile([128, HT, UP], bf16)
            wq = const.tile([128, HT, GW], bf16)
            wk = const.tile([128, HT, GW], bf16)
            wv = const.tile([128, HT, GW], bf16)
            wo = const.tile([128, NHL, H], bf16)
            bq = const.tile([128, NHL], f32)
            bk = const.tile([128, NHL], f32)
            padb = const.tile([128, NKT], f32)
            ones_mat = const.tile([128, 128], bf16)
            nc.vector.memset(ones_mat[:], 1.0)

            r_qg = d_qg[:].rearrange("(t p) u -> p t u", p=128)
            r_y = d_y[:].rearrange("(t p) u -> p t u", p=128)
            r_kuT = d_kuT[:].rearrange("(t p) u -> p t u", p=128)
            r_vuT = d_vuT[:].rearrange("(t p) u -> p t u", p=128)
            r_wq = d_wq[:].rearrange("(t p) d -> p t d", p=128)
            r_wk = d_wk[:].rearrange("(t p) d -> p t d", p=128)
            r_wv = d_wv[:].rearrange("(t p) d -> p t d", p=128)
            r_wo = d_wo[:].rearrange("(n p) h -> p n h", p=128)

            # qg streamed per (head, chunk) just-in-time
            def load_qg(n, ch):
                o, w = qchunks[ch]
                t = qgp.tile([128, HT, 512], bf16, tag="qg", name="qg")
                nc.sync.dma_start(
                    t[:, :, :w], r_qg[:, :, n * UNP + o : n * UNP + o + w]
                )
                return t

            # ---- input DMA ring (single SP HWDGE queue, FIFO) ----
            nc.sync.dma_start(bq[:], d_bq[:])
            nc.sync.dma_start(bk[:], d_bk[:])
            nc.sync.dma_start(padb[:], d_padb[:])
            for ht in range(HT):
                nc.sync.dma_start(kuT[:, ht, :], r_kuT[:, ht, :])
                nc.sync.dma_start(wk[:, ht, :], r_wk[:, ht, :])
            for ht in range(HT):
                nc.sync.dma_start(wq[:, ht, :], r_wq[:, ht, :])
            qg_tiles = {}
            qg_tiles[(0, 0)] = load_qg(0, 0)
            for n in range(NHL):
                nc.sync.dma_start(wo[:, n, :], r_wo[:, n, :])
            for ht in range(HT):
                nc.sync.dma_start(vuT[:, ht, :], r_vuT[:, ht, :])
                nc.sync.dma_start(wv[:, ht, :], r_wv[:, ht, :])
            # remaining qg chunks stream in consumption order below

            bvb = const.tile([128, GW], bf16)
            nc.gpsimd.dma_start(bvb[:], d_bv[0:1, :].to_broadcast([128, GW]))

            def kproj(n):
                ksb = kvp.tile([128, UP], bf16, tag=f"ksb{n}", name=f"ksb{n}")
                for o, w in kchunks:
                    pk = ps_pj.tile([128, 512], f32, tag="pj", name="pk")
                    for ht in range(HT):
                        nc.tensor.matmul(
                            pk[:, :w],
                            wk[:, ht, n * 128 : (n + 1) * 128],
                            kuT[:, ht, o : o + w],
                            start=(ht == 0),
                            stop=(ht == HT - 1),
                        )
                    nc.vector.tensor_scalar_add(
                        ksb[:, o : o + w], pk[:, :w], bk[:, n : n + 1]
                    )
                return ksb

            def vproj():
                vg = vgp.tile([128, NKT, GW], bf16)
                for kt, (ko, klen) in enumerate(ktiles):
                    pv = ps_pj.tile([128, 512], f32, tag="pj", name="pv")
                    for ht in range(HT):
                        nc.tensor.matmul(
                            pv[:klen],
                            vuT[:, ht, ko : ko + klen],
                            wv[:, ht, :],
                            start=(ht == 0),
                            stop=(ht == HT - 1),
                        )
                    nc.vector.tensor_add(vg[:klen, kt, :], pv[:klen], bvb[:klen])
                return vg

            def qproj(n, ch):
                """Project gathered queries for (head n, chunk ch)."""
                o, w = qchunks[ch]
                qgt = qg_tiles.pop((n, ch))
                qsb = qsp.tile([128, 512], bf16, tag="qsb", name="qsb")
                pq = ps_pj.tile([128, 512], f32, tag="pj", name="pq")
                for ht in range(HT):
                    nc.tensor.matmul(
                        pq[:, :w],
                        wq[:, ht, n * 128 : (n + 1) * 128],
                        qgt[:, ht, :w],
                        start=(ht == 0),
                        stop=(ht == HT - 1),
                    )
                nc.vector.tensor_scalar_add(qsb[:, :w], pq[:, :w], bq[:, n : n + 1])
                return qsb

            def attention(n, ch, ksb, qsb, vg):
                """Attention + per-head output projection for one q-chunk."""
                o, w = qchunks[ch]
                ppv = ps_pv.tile([128, 512], f32, name="ppv")
                # esum split into two independent partial chains: low kt
                # tiles accumulate on DVE, high kt tiles on the (otherwise
                # idle) gpsimd engine; one DVE add merges them. Only the
                # first klen partitions of each e are valid — slice by klen
                # so the short last tile doesn't pull in stale pool data.
                esum = esp.tile([128, 512], bf16, tag="esum", name="esum")
                esumB = esp.tile([128, 512], bf16, tag="esumB", name="esumB")
                kt_split = (NKT + 1) // 2  # group0: [0, split), group1: rest
                # group1's first two tiles must be full-width (the only
                # partial tile is the last one)
                assert kt_split >= 2 and NKT - kt_split >= 3
                etiles = []
                for kt, (ko, klen) in enumerate(ktiles):
                    ps = ps_sc.tile([128, 512], f32, tag="sc", name="ps")
                    nc.tensor.matmul(
                        ps[:klen, :w],
                        ksb[:, ko : ko + klen],
                        qsb[:, :w],
                        start=True,
                        stop=True,
                    )
                    e = expp.tile([128, 512], bf16, name="e")
                    nc.scalar.activation(
                        out=e[:klen, :w],
                        in_=ps[:klen, :w],
                        func=mybir.ActivationFunctionType.Exp,
                        bias=padb[:klen, kt : kt + 1],
                        scale=SCALE,
                    )
                    nc.tensor.matmul(
                        ppv[:, :w],
                        vg[:klen, kt, n * 128 : (n + 1) * 128],
                        e[:klen, :w],
                        start=(kt == 0),
                        stop=(kt == NKT - 1),
                    )
                    etiles.append((e, klen))
                    grp = 0 if kt < kt_split else 1
                    acc, eng = (esum, nc.vector) if grp == 0 else (esumB, nc.gpsimd)
                    first = 0 if grp == 0 else kt_split
                    if kt == first:
                        pass
                    elif kt == first + 1:
                        ep, kp = etiles[first]
                        kl = min(klen, kp)
                        eng.tensor_add(acc[:kl, :w], ep[:kl, :w], e[:kl, :w])
                    else:
                        eng.tensor_add(acc[:klen, :w], acc[:klen, :w], e[:klen, :w])
                nc.vector.tensor_add(esum[:, :w], esum[:, :w], esumB[:, :w])
                pd = ps_pd.tile([128, 512], f32, name="pd")
                nc.tensor.matmul(pd[:, :w], ones_mat[:], esum[:, :w], start=True, stop=True)
                rec = scp.tile([128, 512], f32, tag="rec", name="rec")
                nc.vector.reciprocal_approx_fast(out=rec[:, :w], in_=pd[:, :w])
                oc = ocp.tile([128, 512], bf16, name="oc")
                nc.vector.tensor_mul(oc[:, :w], ppv[:, :w], rec[:, :w])

                # per-head output projection for this chunk
                yt = ytp.tile([128, HT, 512], bf16, name="yt")
                for ht in range(HT):
                    py = ps_oj.tile([128, 512], f32, tag="oj", name="py")
                    nc.tensor.matmul(
                        py[:, :w],
                        wo[:, n, ht * 128 : (ht + 1) * 128],
                        oc[:, :w],
                        start=True,
                        stop=True,
                    )
                    if ht in (2, 5, 7):
                        nc.vector.tensor_scalar_add(yt[:, ht, :w], py[:, :w], 0.0)
                    else:
                        nc.scalar.copy(yt[:, ht, :w], py[:, :w])
                nc.scalar.dma_start(
                    r_y[:, :, n * UNP + o : n * UNP + o + w], yt[:, :, :w]
                )

            # ---- emission ----
            ks = [kproj(n) for n in range(NHL)]
            qs0 = qproj(0, 0)
            vg = vproj()
            # chunk-major order: the cheap tail chunk (short, if any) runs
            # last across all heads, shrinking the end-of-kernel drain
            units = [(n, ch) for ch in range(NQC) for n in range(NHL)]
            qsbs = {(0, 0): qs0}
            # stream remaining qg loads in consumption order on the ring
            for u in units[1:]:
                qg_tiles[u] = load_qg(*u)
            for i, (n, ch) in enumerate(units):
                # prefetch the NEXT unit's q-projection BEFORE this unit's
                # attention so its scores can overlap this unit's finalize
                # and output projection
                if i + 1 < len(units):
                    nxt = units[i + 1]
                    qsbs[nxt] = qproj(*nxt)
                attention(n, ch, ks[n], qsbs.pop((n, ch)), vg)

    nc.compile()
    return nc


def _prepare(query, key, value, key_padding_mask, Wq, bq, Wk, bk, Wv, bv, Wo, bo):
    """Host-side prep: mask constants (fp64), gathers/transposes, per-core maps."""
    mask = np.asarray(key_padding_mask)
    q64 = np.asarray(query, np.float64)
    Wq64 = np.asarray(Wq, np.float64)
    Wk64 = np.asarray(Wk, np.float64)
    Wv64 = np.asarray(Wv, np.float64)
    Wo64 = np.asarray(Wo, np.float64)

    # shared projected row of all masked keys, per head
    kmask = NEG * Wk64.sum(axis=1) + np.asarray(bk, np.float64)  # [NH, DK]

    # z sign per (s, b, n):  z = q . (Wq[n] @ kmask[n]) + bq[n].kmask[n]
    wz = np.einsum("nhd,nd->hn", Wq64, kmask)  # [H, NH]
    cz = np.einsum("nd,nd->n", np.asarray(bq, np.float64), kmask)  # [NH]
    z = q64.reshape(S * B, H) @ wz + cz  # [S*B, NH]
    choose = (z > 0).reshape(S, B, NH)

    # mask-branch output: mean of (unmasked-data) V over masked key positions
    v64 = np.asarray(value, np.float64)  # [S, B, H]
    vbar_feat = np.stack(
        [
            v64[mask[b], b, :].mean(axis=0) if mask[b].any() else np.zeros(H)
            for b in range(B)
        ]
    )  # [B, H]
    for b in range(B):
        if not mask[b].any():
            choose[:, b, :] = False  # no masked keys -> no mask branch
        elif mask[b].all():
            # all keys masked: identical scores -> uniform softmax -> Vbar
            choose[:, b, :] = True
    vbar = (
        np.einsum("bh,nhd->bnd", vbar_feat, Wv64) + np.asarray(bv, np.float64)[None]
    )  # [B, NH, DK]
    ubar = np.einsum("bnd,ndh->bnh", vbar, Wo64.reshape(NH, DK, H))  # [B, NH, H]

    # correction added on host for mask-branch rows
    ycorr = np.einsum("sbn,bnh->sbh", choose.astype(np.float64), ubar)

    # gather unmasked keys per batch
    idx = [np.nonzero(~mask[b])[0] for b in range(B)]
    UP = max(max(len(i) for i in idx), 1)
    NKT = (UP + 127) // 128

    # gather not-chosen queries per (batch, head)
    qidx = [[np.nonzero(~choose[:, b, n])[0] for n in range(NH)] for b in range(B)]
    UNP = max(max(len(qidx[b][n]) for n in range(NH)) for b in range(B))
    UNP = max(UNP, 1)

    Wq_np = np.asarray(Wq, np.float32)
    Wk_np = np.asarray(Wk, np.float32)
    Wv_np = np.asarray(Wv, np.float32)
    Wo_np = np.asarray(Wo, np.float32)
    bq_np = np.asarray(bq, np.float32)
    bk_np = np.asarray(bk, np.float32)
    bv_np = np.asarray(bv, np.float32)
    bo_np = np.asarray(bo, np.float32)

    base_padb = np.zeros((128, NKT), np.float32)
    flat = np.arange(NKT * 128).reshape(NKT, 128).T  # [128, NKT] key index

    q_np = np.asarray(query, np.float32)

    kuT_b, vuT_b, padb_b = [], [], []
    for b in range(B):
        ii = idx[b]
        u = len(ii)
        kuT = np.zeros((H, UP), npbf16)
        kuT[:, :u] = np.asarray(key[ii, b, :], np.float32).T.astype(npbf16)
        vuT = np.zeros((H, UP), npbf16)
        vuT[:, :u] = np.asarray(value[ii, b, :], np.float32).T.astype(npbf16)
        pb = base_padb.copy()
        pb[flat >= max(u, 1)] = -30000.0  # keep >=1 live key (denom > 0)
        kuT_b.append(kuT)
        vuT_b.append(vuT)
        padb_b.append(pb)

    in_maps = []
    for core in range(NCORES):
        b, g = divmod(core, 2)
        hsl = slice(g * NHL, (g + 1) * NHL)
        qg = np.zeros((H, NHL * UNP), npbf16)
        for nl in range(NHL):
            qi = qidx[b][g * NHL + nl]
            qg[:, nl * UNP : nl * UNP + len(qi)] = (
                q_np[qi, b, :].T.astype(npbf16)
            )
        wq_d = np.ascontiguousarray(
            Wq_np[hsl].transpose(1, 0, 2).reshape(H, GW)
        ).astype(npbf16)
        wk_d = np.ascontiguousarray(
            Wk_np[hsl].transpose(1, 0, 2).reshape(H, GW)
        ).astype(npbf16)
        wv_d = np.ascontiguousarray(
            Wv_np[hsl].transpose(1, 0, 2).reshape(H, GW)
        ).astype(npbf16)
        wo_d = np.ascontiguousarray(Wo_np[g * GW : (g + 1) * GW, :]).astype(npbf16)
        bq_d = np.ascontiguousarray(bq_np[hsl].T)  # [DK, NHL]
        bk_d = np.ascontiguousarray(bk_np[hsl].T)
        bv_d = bv_np[hsl].reshape(1, GW).astype(npbf16)
        in_maps.append(
            {
                "qg": qg,
                "kuT": kuT_b[b],
                "vuT": vuT_b[b],
                "wq": wq_d,
                "wk": wk_d,
                "wv": wv_d,
                "wo": wo_d,
                "bq": bq_d,
                "bk": bk_d,
                "bv": bv_d,
                "padb": padb_b[b],
            }
        )
    return in_maps, ycorr + bo_np[None, None, :], UP, UNP, qidx


def run(inputs: dict, trace: bool = False):
    in_maps, ycorr, UP, UNP, qidx = _prepare(**inputs)
    key_ = (UP, UNP)
    if key_ not in _PROG_CACHE:
        _PROG_CACHE[key_] = build_program(UP, UNP)
    nc = _PROG_CACHE[key_]
    res = run_bass_kernel_spmd(nc, in_maps, list(range(NCORES)), trace=trace)
    y = np.ascontiguousarray(ycorr.astype(np.float32))  # [S, B, H], incl. bo
    for core in range(NCORES):
        b, g = divmod(core, 2)
        yp = res.results[core]["y"].astype(np.float32)  # [H, NHL*UNP]
        for nl in range(NHL):
            qi = qidx[b][g * NHL + nl]
            if len(qi):
                y[qi, b, :] += yp[:, nl * UNP : nl * UNP + len(qi)].T
    return y, res


def kernel(**inputs) -> np.ndarray:
    y, _ = run(inputs, trace=False)
    return y


# revision 36
# speedup vs baseline: 1.1447x; 1.0842x over previous
"""Trainium2 Bass kernel for nn_MultiHeadHCGAttention.

Math notes (exact restructuring of the reference):
  The key_padding_mask replaces the ENTIRE key feature row with -1e9 BEFORE
  the K projection (v is NOT masked). Hence every masked key position s in
  batch b has the SAME projected K row:
      Kmask[n] = -1e9 * sum_h Wk[n,h,:] + bk[n]   (data independent)
  All masked keys share one score z = Q.Kmask/sqrt(dk) with |z| ~ 1e9.
  In fp32 softmax the output per (query q, head n) is therefore either
    - mean of V over the masked key positions  if z > max unmasked score
      (uniform softmax over the identical-score masked keys)
    - standard softmax over unmasked keys      otherwise (masked weights
      underflow to exactly 0 in fp32)
  The boundary band has probability ~1e-7 per query -> decided by sign(z),
  computed exactly on the host in fp64 (z = q @ (Wq@Kmask) + bq.Kmask).

  KEY SPARSITY EXPLOITED ON DEVICE: ~50% of (query, head) pairs take the
  mask branch; their output is entirely host-computed (ubar via ycorr).
  The device only processes, per head, the GATHERED not-chosen queries
  (like the keys are gathered per batch) — roughly halving all
  query-dimension work (q-projection, scores, exp, PV, out-projection).
  Because the gathered query sets differ per head, the output projection
  is per-head; the host scatter-adds the per-head partials into the full
  output (and adds bo + ycorr in fp64/fp32).

Sharding: 8 cores = (batch b in 0..3) x (head group of 4). No collectives.

Softmax denominator: e-tiles are accumulated over key-tiles on the vector
engine (esum), one ones-matmul per (head, query-chunk) on the PE, and
1/d via reciprocal_approx_fast (~18-bit).
"""

import math
import sys

if "/opt/trn_rl_repo" not in sys.path:
    sys.path.insert(0, "/opt/trn_rl_repo")

import ml_dtypes
import numpy as np

import concourse.bacc as bacc
import concourse.tile as tile
from concourse import mybir
from concourse.bass_utils import run_bass_kernel_spmd

S, B, H = 2048, 4, 1024
NH, DK = 8, 128
NHDK = NH * DK
NHL = 4            # heads per core (head group)
GW = NHL * DK      # 512: projection width per core
NEG = -1.0e9
NCORES = 8
HT = H // 128      # 8 H-tiles

bf16 = mybir.dt.bfloat16
f32 = mybir.dt.float32
npbf16 = ml_dtypes.bfloat16

_PROG_CACHE: dict = {}


def _chunks(total, step=512):
    out = []
    o = 0
    while o < total:
        w = min(step, total - o)
        out.append((o, w))
        o += w
    return out


def build_program(UP: int, UNP: int):
    """Per-core SPMD program.

    UP  = gathered unmasked-key count (global max over batches)
    UNP = gathered not-chosen query slot width per head (global max)
    """
    NKT = (UP + 127) // 128
    ktiles = [(o, min(128, UP - o)) for o in range(0, UP, 128)]
    kchunks = _chunks(UP)
    qchunks = _chunks(UNP)
    NQC = len(qchunks)
    assert NKT >= 3  # esum init uses tiles kt=0,1 at full width

    nc = bacc.Bacc("TRN2", target_bir_lowering=False, debug=False)

    d_qg = nc.dram_tensor("qg", [H, NHL * UNP], bf16, kind="ExternalInput")
    d_kuT = nc.dram_tensor("kuT", [H, UP], bf16, kind="ExternalInput")
    d_vuT = nc.dram_tensor("vuT", [H, UP], bf16, kind="ExternalInput")
    d_wq = nc.dram_tensor("wq", [H, GW], bf16, kind="ExternalInput")
    d_wk = nc.dram_tensor("wk", [H, GW], bf16, kind="ExternalInput")
    d_wv = nc.dram_tensor("wv", [H, GW], bf16, kind="ExternalInput")
    d_wo = nc.dram_tensor("wo", [GW, H], bf16, kind="ExternalInput")
    d_bq = nc.dram_tensor("bq", [DK, NHL], f32, kind="ExternalInput")
    d_bk = nc.dram_tensor("bk", [DK, NHL], f32, kind="ExternalInput")
    d_bv = nc.dram_tensor("bv", [1, GW], bf16, kind="ExternalInput")
    d_padb = nc.dram_tensor("padb", [128, NKT], f32, kind="ExternalInput")
    d_y = nc.dram_tensor("y", [H, NHL * UNP], bf16, kind="ExternalOutput")

    SCALE = 1.0 / math.sqrt(DK)

    with tile.TileContext(nc) as tc:
        with (
            tc.tile_pool(name="const", bufs=1) as const,
            tc.tile_pool(name="kv", bufs=1) as kvp,
            tc.tile_pool(name="qg", bufs=3) as qgp,
            tc.tile_pool(name="qs", bufs=3) as qsp,
            tc.tile_pool(name="vg", bufs=1) as vgp,
            tc.tile_pool(name="exp", bufs=12) as expp,
            tc.tile_pool(name="es", bufs=2) as esp,
            tc.tile_pool(name="sc", bufs=2) as scp,
            tc.tile_pool(name="oc", bufs=3) as ocp,
            tc.tile_pool(name="yt", bufs=3) as ytp,
            tc.tile_pool(name="ps_pj", bufs=2, space="PSUM") as ps_pj,
            tc.tile_pool(name="ps_oj", bufs=1, space="PSUM") as ps_oj,
            tc.tile_pool(name="ps_sc", bufs=2, space="PSUM") as ps_sc,
            tc.tile_pool(name="ps_pv", bufs=2, space="PSUM") as ps_pv,
            tc.tile_pool(name="ps_pd", bufs=1, space="PSUM") as ps_pd,
        ):
            # ---- SBUF residents ----
            kuT = const.tile([128, HT, UP], bf16)
            vuT = const.tile([128, HT, UP], bf16)
            wq = const.tile([128, HT, GW], bf16)
            wk = const.tile([128, HT, GW], bf16)
            wv = const.tile([128, HT, GW], bf16)
            wo = const.tile([128, NHL, H], bf16)
            bq = const.tile([128, NHL], f32)
            bk = const.tile([128, NHL], f32)
            padb = const.tile([128, NKT], f32)
            ones_mat = const.tile([128, 128], bf16)
            nc.vector.memset(ones_mat[:], 1.0)

            r_qg = d_qg[:].rearrange("(t p) u -> p t u", p=128)
            r_y = d_y[:].rearrange("(t p) u -> p t u", p=128)
            r_kuT = d_kuT[:].rearrange("(t p) u -> p t u", p=128)
            r_vuT = d_vuT[:].rearrange("(t p) u -> p t u", p=128)
            r_wq = d_wq[:].rearrange("(t p) d -> p t d", p=128)
            r_wk = d_wk[:].rearrange("(t p) d -> p t d", p=128)
            r_wv = d_wv[:].rearrange("(t p) d -> p t d", p=128)
            r_wo = d_wo[:].rearrange("(n p) h -> p n h", p=128)

            # qg streamed per (head, chunk) just-in-time
            def load_qg(n, ch):
                o, w = qchunks[ch]
                t = qgp.tile([128, HT, 512], bf16, tag="qg", name="qg")
                nc.sync.dma_start(
                    t[:, :, :w], r_qg[:, :, n * UNP + o : n * UNP + o + w]
                )
                return t

            # ---- input DMA ring (single SP HWDGE queue, FIFO) ----
            nc.sync.dma_start(bq[:], d_bq[:])
            nc.sync.dma_start(bk[:], d_bk[:])
            nc.sync.dma_start(padb[:], d_padb[:])
            for ht in range(HT):
                nc.sync.dma_start(kuT[:, ht, :], r_kuT[:, ht, :])
                nc.sync.dma_start(wk[:, ht, :], r_wk[:, ht, :])
            # unit order: ascending chunk width, so the cheap short-chunk
            # units run during the DMA-bound ramp and the dense units
            # pipeline through the steady state
            ch_order = sorted(range(NQC), key=lambda c: qchunks[c][1])
            units = [(n, ch) for ch in ch_order for n in range(NHL)]

            for ht in range(HT):
                nc.sync.dma_start(wq[:, ht, :], r_wq[:, ht, :])
            qg_tiles = {}
            for u in units[:2]:
                qg_tiles[u] = load_qg(*u)
            for n in range(NHL):
                nc.sync.dma_start(wo[:, n, :], r_wo[:, n, :])
            for ht in range(HT):
                nc.sync.dma_start(vuT[:, ht, :], r_vuT[:, ht, :])
                nc.sync.dma_start(wv[:, ht, :], r_wv[:, ht, :])
            # remaining qg chunks stream just-in-time inside the unit loop

            bvb = const.tile([128, GW], bf16)
            nc.gpsimd.dma_start(bvb[:], d_bv[0:1, :].to_broadcast([128, GW]))

            def kproj(n):
                ksb = kvp.tile([128, UP], bf16, tag=f"ksb{n}", name=f"ksb{n}")
                for o, w in kchunks:
                    pk = ps_pj.tile([128, 512], f32, tag="pj", name="pk")
                    for ht in range(HT):
                        nc.tensor.matmul(
                            pk[:, :w],
                            wk[:, ht, n * 128 : (n + 1) * 128],
                            kuT[:, ht, o : o + w],
                            start=(ht == 0),
                            stop=(ht == HT - 1),
                        )
                    nc.vector.tensor_scalar_add(
                        ksb[:, o : o + w], pk[:, :w], bk[:, n : n + 1]
                    )
                return ksb

            def vproj():
                vg = vgp.tile([128, NKT, GW], bf16)
                for kt, (ko, klen) in enumerate(ktiles):
                    pv = ps_pj.tile([128, 512], f32, tag="pj", name="pv")
                    for ht in range(HT):
                        nc.tensor.matmul(
                            pv[:klen],
                            vuT[:, ht, ko : ko + klen],
                            wv[:, ht, :],
                            start=(ht == 0),
                            stop=(ht == HT - 1),
                        )
                    nc.vector.tensor_add(vg[:klen, kt, :], pv[:klen], bvb[:klen])
                return vg

            def qproj(n, ch):
                """Project gathered queries for (head n, chunk ch)."""
                o, w = qchunks[ch]
                qgt = qg_tiles.pop((n, ch))
                qsb = qsp.tile([128, 512], bf16, tag="qsb", name="qsb")
                pq = ps_pj.tile([128, 512], f32, tag="pj", name="pq")
                for ht in range(HT):
                    nc.tensor.matmul(
                        pq[:, :w],
                        wq[:, ht, n * 128 : (n + 1) * 128],
                        qgt[:, ht, :w],
                        start=(ht == 0),
                        stop=(ht == HT - 1),
                    )
                nc.vector.tensor_scalar_add(qsb[:, :w], pq[:, :w], bq[:, n : n + 1])
                return qsb

            def attention(n, ch, ksb, qsb, vg):
                """Attention + per-head output projection for one q-chunk."""
                o, w = qchunks[ch]
                ppv = ps_pv.tile([128, 512], f32, name="ppv")
                # esum split into two independent partial chains: EARLY kt
                # tiles accumulate on the slow-but-idle gpsimd engine (its
                # latency hides under the rest of the unit), LATE kt tiles
                # on DVE, and the DVE also merges — so the critical tail
                # (last exp -> add -> merge -> ones-mm) stays on one fast
                # queue. Only the first klen partitions of each e are valid
                # — slice by klen so the short last tile doesn't pull in
                # stale pool data.
                esum = esp.tile([128, 512], bf16, tag="esum", name="esum")
                esumB = esp.tile([128, 512], bf16, tag="esumB", name="esumB")
                kt_split = (NKT + 1) // 2  # group0: [0, split), group1: rest
                # group1's first two tiles must be full-width (the only
                # partial tile is the last one)
                assert kt_split >= 2 and NKT - kt_split >= 3
                etiles = []
                for kt, (ko, klen) in enumerate(ktiles):
                    ps = ps_sc.tile([128, 512], f32, tag="sc", name="ps")
                    nc.tensor.matmul(
                        ps[:klen, :w],
                        ksb[:, ko : ko + klen],
                        qsb[:, :w],
                        start=True,
                        stop=True,
                    )
                    e = expp.tile([128, 512], bf16, name="e")
                    nc.scalar.activation(
                        out=e[:klen, :w],
                        in_=ps[:klen, :w],
                        func=mybir.ActivationFunctionType.Exp,
                        bias=padb[:klen, kt : kt + 1],
                        scale=SCALE,
                    )
                    nc.tensor.matmul(
                        ppv[:, :w],
                        vg[:klen, kt, n * 128 : (n + 1) * 128],
                        e[:klen, :w],
                        start=(kt == 0),
                        stop=(kt == NKT - 1),
                    )
                    etiles.append((e, klen))
                    grp = 0 if kt < kt_split else 1
                    acc, eng = (esum, nc.gpsimd) if grp == 0 else (esumB, nc.vector)
                    first = 0 if grp == 0 else kt_split
                    if kt == first:
                        pass
                    elif kt == first + 1:
                        ep, kp = etiles[first]
                        kl = min(klen, kp)
                        eng.tensor_add(acc[:kl, :w], ep[:kl, :w], e[:kl, :w])
                    else:
                        eng.tensor_add(acc[:klen, :w], acc[:klen, :w], e[:klen, :w])
                nc.vector.tensor_add(esum[:, :w], esum[:, :w], esumB[:, :w])
                pd = ps_pd.tile([128, 512], f32, name="pd")
                nc.tensor.matmul(pd[:, :w], ones_mat[:], esum[:, :w], start=True, stop=True)
                rec = scp.tile([128, 512], f32, tag="rec", name="rec")
                nc.vector.reciprocal_approx_fast(out=rec[:, :w], in_=pd[:, :w])
                oc = ocp.tile([128, 512], bf16, name="oc")
                nc.vector.tensor_mul(oc[:, :w], ppv[:, :w], rec[:, :w])

                # per-head output projection for this chunk
                yt = ytp.tile([128, HT, 512], bf16, name="yt")
                for ht in range(HT):
                    py = ps_oj.tile([128, 512], f32, tag="oj", name="py")
                    nc.tensor.matmul(
                        py[:, :w],
                        wo[:, n, ht * 128 : (ht + 1) * 128],
                        oc[:, :w],
                        start=True,
                        stop=True,
                    )
                    if ht in (2, 5, 7):
                        nc.vector.tensor_scalar_add(yt[:, ht, :w], py[:, :w], 0.0)
                    else:
                        nc.scalar.copy(yt[:, ht, :w], py[:, :w])
                nc.sync.dma_start(
                    r_y[:, :, n * UNP + o : n * UNP + o + w], yt[:, :, :w]
                )

            # ---- emission ----
            ks = [kproj(n) for n in range(NHL)]
            qsbs = {units[0]: qproj(*units[0])}
            vg = vproj()
            for i, (n, ch) in enumerate(units):
                # stream the qg load two units ahead (ring position after
                # the previous unit's output DMA; pool bufs keep it honest)
                if i + 2 < len(units):
                    qg_tiles[units[i + 2]] = load_qg(*units[i + 2])
                # prefetch the NEXT unit's q-projection BEFORE this unit's
                # attention so its scores can overlap this unit's finalize
                # and output projection
                if i + 1 < len(units):
                    nxt = units[i + 1]
                    qsbs[nxt] = qproj(*nxt)
                attention(n, ch, ks[n], qsbs.pop((n, ch)), vg)

    nc.compile()
    return nc


def _prepare(query, key, value, key_padding_mask, Wq, bq, Wk, bk, Wv, bv, Wo, bo):
    """Host-side prep: mask constants (fp64), gathers/transposes, per-core maps."""
    mask = np.asarray(key_padding_mask)
    q64 = np.asarray(query, np.float64)
    Wq64 = np.asarray(Wq, np.float64)
    Wk64 = np.asarray(Wk, np.float64)
    Wv64 = np.asarray(Wv, np.float64)
    Wo64 = np.asarray(Wo, np.float64)

    # shared projected row of all masked keys, per head
    kmask = NEG * Wk64.sum(axis=1) + np.asarray(bk, np.float64)  # [NH, DK]

    # z sign per (s, b, n):  z = q . (Wq[n] @ kmask[n]) + bq[n].kmask[n]
    wz = np.einsum("nhd,nd->hn", Wq64, kmask)  # [H, NH]
    cz = np.einsum("nd,nd->n", np.asarray(bq, np.float64), kmask)  # [NH]
    z = q64.reshape(S * B, H) @ wz + cz  # [S*B, NH]
    choose = (z > 0).reshape(S, B, NH)

    # mask-branch output: mean of (unmasked-data) V over masked key positions
    v64 = np.asarray(value, np.float64)  # [S, B, H]
    vbar_feat = np.stack(
        [
            v64[mask[b], b, :].mean(axis=0) if mask[b].any() else np.zeros(H)
            for b in range(B)
        ]
    )  # [B, H]
    for b in range(B):
        if not mask[b].any():
            choose[:, b, :] = False  # no masked keys -> no mask branch
        elif mask[b].all():
            # all keys masked: identical scores -> uniform softmax -> Vbar
            choose[:, b, :] = True
    vbar = (
        np.einsum("bh,nhd->bnd", vbar_feat, Wv64) + np.asarray(bv, np.float64)[None]
    )  # [B, NH, DK]
    ubar = np.einsum("bnd,ndh->bnh", vbar, Wo64.reshape(NH, DK, H))  # [B, NH, H]

    # correction added on host for mask-branch rows
    ycorr = np.einsum("sbn,bnh->sbh", choose.astype(np.float64), ubar)

    # gather unmasked keys per batch
    idx = [np.nonzero(~mask[b])[0] for b in range(B)]
    UP = max(max(len(i) for i in idx), 1)
    NKT = (UP + 127) // 128

    # gather not-chosen queries per (batch, head)
    qidx = [[np.nonzero(~choose[:, b, n])[0] for n in range(NH)] for b in range(B)]
    UNP = max(max(len(qidx[b][n]) for n in range(NH)) for b in range(B))
    UNP = max(UNP, 1)

    Wq_np = np.asarray(Wq, np.float32)
    Wk_np = np.asarray(Wk, np.float32)
    Wv_np = np.asarray(Wv, np.float32)
    Wo_np = np.asarray(Wo, np.float32)
    bq_np = np.asarray(bq, np.float32)
    bk_np = np.asarray(bk, np.float32)
    bv_np = np.asarray(bv, np.float32)
    bo_np = np.asarray(bo, np.float32)

    base_padb = np.zeros((128, NKT), np.float32)
    flat = np.arange(NKT * 128).reshape(NKT, 128).T  # [128, NKT] key index

    q_np = np.asarray(query, np.float32)

    kuT_b, vuT_b, padb_b = [], [], []
    for b in range(B):
        ii = idx[b]
        u = len(ii)
        kuT = np.zeros((H, UP), npbf16)
        kuT[:, :u] = np.asarray(key[ii, b, :], np.float32).T.astype(npbf16)
        vuT = np.zeros((H, UP), npbf16)
        vuT[:, :u] = np.asarray(value[ii, b, :], np.float32).T.astype(npbf16)
        pb = base_padb.copy()
        pb[flat >= max(u, 1)] = -30000.0  # keep >=1 live key (denom > 0)
        kuT_b.append(kuT)
        vuT_b.append(vuT)
        padb_b.append(pb)

    in_maps = []
    for core in range(NCORES):
        b, g = divmod(core, 2)
        hsl = slice(g * NHL, (g + 1) * NHL)
        qg = np.zeros((H, NHL * UNP), npbf16)
        for nl in range(NHL):
            qi = qidx[b][g * NHL + nl]
            qg[:, nl * UNP : nl * UNP + len(qi)] = (
                q_np[qi, b, :].T.astype(npbf16)
            )
        wq_d = np.ascontiguousarray(
            Wq_np[hsl].transpose(1, 0, 2).reshape(H, GW)
        ).astype(npbf16)
        wk_d = np.ascontiguousarray(
            Wk_np[hsl].transpose(1, 0, 2).reshape(H, GW)
        ).astype(npbf16)
        wv_d = np.ascontiguousarray(
            Wv_np[hsl].transpose(1, 0, 2).reshape(H, GW)
        ).astype(npbf16)
        wo_d = np.ascontiguousarray(Wo_np[g * GW : (g + 1) * GW, :]).astype(npbf16)
        bq_d = np.ascontiguousarray(bq_np[hsl].T)  # [DK, NHL]
        bk_d = np.ascontiguousarray(bk_np[hsl].T)
        bv_d = bv_np[hsl].reshape(1, GW).astype(npbf16)
        in_maps.append(
            {
                "qg": qg,
                "kuT": kuT_b[b],
                "vuT": vuT_b[b],
                "wq": wq_d,
                "wk": wk_d,
                "wv": wv_d,
                "wo": wo_d,
                "bq": bq_d,
                "bk": bk_d,
                "bv": bv_d,
                "padb": padb_b[b],
            }
        )
    return in_maps, ycorr + bo_np[None, None, :], UP, UNP, qidx


def run(inputs: dict, trace: bool = False):
    in_maps, ycorr, UP, UNP, qidx = _prepare(**inputs)
    key_ = (UP, UNP)
    if key_ not in _PROG_CACHE:
        _PROG_CACHE[key_] = build_program(UP, UNP)
    nc = _PROG_CACHE[key_]
    res = run_bass_kernel_spmd(nc, in_maps, list(range(NCORES)), trace=trace)
    y = np.ascontiguousarray(ycorr.astype(np.float32))  # [S, B, H], incl. bo
    for core in range(NCORES):
        b, g = divmod(core, 2)
        yp = res.results[core]["y"].astype(np.float32)  # [H, NHL*UNP]
        for nl in range(NHL):
            qi = qidx[b][g * NHL + nl]
            if len(qi):
                y[qi, b, :] += yp[:, nl * UNP : nl * UNP + len(qi)].T
    return y, res


def kernel(**inputs) -> np.ndarray:
    y, _ = run(inputs, trace=False)
    return y
